# revision 56
# baseline (speedup 1.0000x reference)
"""Trainium2 Bass kernel for CustomMamba2D.

Sharding: data-parallel over batch across 8 NeuronCores (B=8 -> 1 image/core).

Per-core layout: the 64ch x 512x512 image is processed in 128 row-QUADS.
A quad tile is [128 partitions, 1024 free]: partitions = (row-in-pair r,
channel c) = 2*64, free = (pair-in-quad rp, w) = 2*512.  All channel-mixing
matmuls use 128-partition operands with block-diagonal weights.  The
depthwise 3x3 conv is 18 PE matmuls per quad with diagonal weight blocks;
the 12 "cross" row taps use 64-partition operands packed into disjoint PE
quadrants via tile_position.  dx shifts are free-dim AP offsets (narrower
accumulation range = zero pad).

SSM shortcut: exp(A*k) underflows to exactly 0.0 (fp32) for k >= ~1040, so
the reference cumsum is bitwise constant from l = 2048 on.  Only rows 0..3
need the scan (DVE tensor_tensor_scan); all other positions use the
constant c* = wC @ state[:, 2047].

sigmoid(g) = (tanh(g/2)+1)/2 with the 0.5 folded into the gate weights and
w_out, so ALL transcendentals (SiLU x2, tanh) live in one ACT table set
(silu_and_others -> one table load).  BN scales fold into weights host-side.
Intermediates are fp16 (DVE 2x/4x modes); matmuls on fp32 data use float32r
(1 cycle/col at free-dim >= 256).  Output DMAs ride the otherwise-idle
GPSIMD SWDGE queue so the SP sequencer only pays for input DMAs.
"""

import os
import sys

for _p in (
    "/root/.axon_site",
    "/root/.axon_site/_ro/trn_rl_repo",
    "/root/.axon_site/_ro/pypackages",
    "/opt/trn_rl_repo",
    "/opt/pypackages",
):
    if os.path.isdir(_p) and _p not in sys.path:
        sys.path.append(_p)

import numpy as np

import concourse.bass as bass
import concourse.mybir as mybir
from concourse.tile import TileContext
from concourse import bass_utils

f32 = mybir.dt.float32
f32r = mybir.dt.float32r
f16 = mybir.dt.float16
AF = mybir.ActivationFunctionType
ALU = mybir.AluOpType

D_MODEL, D_STATE, D_CONV, D_INNER = 64, 16, 3, 64
BN_EPS = 1e-5
B, H, W = 8, 512, 512
NQ = H // 4            # 128 row quads (4 rows each)
HEAD_L = 2048          # rows 0..3 carry the live part of the scan
N_CORES = 8

# (out_slice, in_slice) per kx = dx+1, within one 512-wide pair block
_SH = (
    (slice(1, W), slice(0, W - 1)),
    (slice(0, W), slice(0, W)),
    (slice(0, W - 1), slice(1, W)),
)


def _off(sl, base):
    return slice(sl.start + base, sl.stop + base)


def _split_waits(nc, maxw=1):
    """This walrus build encodes at most ONE sync wait per instruction and
    refuses to split multi-wait instructions itself.  Move extra waits onto
    NoOp carriers inserted just before the owning instruction (same engine,
    so ordering is preserved)."""
    for fn in nc.m.functions:
        for bb in fn.blocks:
            out, changed = [], False
            for inst in bb.instructions:
                si = inst.sync_info
                if si is not None and len(si.on_wait) > maxw:
                    waits = list(si.on_wait)
                    for k, wt in enumerate(waits[maxw:]):
                        out.append(
                            mybir.InstNoOp(
                                name=f"{inst.name}_sw{k}",
                                engine=inst.engine,
                                bass_nofuse=True,
                                sync_info=mybir.SyncInfo(on_wait=[wt], on_update=[]),
                            )
                        )
                    inst.sync_info = mybir.SyncInfo(
                        on_wait=waits[:maxw], on_update=list(si.on_update)
                    )
                    changed = True
                out.append(inst)
            if changed:
                bb.instructions = out


# PSUM/schedule configuration (sweepable via TimelineSim):
#   split_tanh: two [128,512] gate psums + two tanh ops instead of one
#       merged [128,1024] psum + single tanh
#   ybias_lag: iterations between out-proj MM and the ybias DVE read
_CFG = dict(split_tanh=True, pin=1, pot=1, pdw=3, psig=1, ptau=2, ybias_lag=0)


def _build_program(zero_gate_bias):
    nc = bass.Bass("TRN2", target_bir_lowering=False, debug=False, num_devices=N_CORES)

    x_d = nc.dram_tensor("x", [D_MODEL, H, W], f32r, kind="ExternalInput")
    y_d = nc.dram_tensor("y", [D_MODEL, H, W], f32, kind="ExternalOutput")
    lin_d = nc.dram_tensor("lhsT_in", [128, 128], f32r, kind="ExternalInput")
    ldw_d = nc.dram_tensor("lhsT_dw", [3, 128, 128], f16, kind="ExternalInput")
    lcx_d = nc.dram_tensor("lhsT_cx", [3, 128, 64], f16, kind="ExternalInput")
    lgl_d = nc.dram_tensor("lhsT_gl", [128, 128], f16, kind="ExternalInput")
    lgh_d = nc.dram_tensor("lhsT_gh", [128, 128], f16, kind="ExternalInput")
    lo_d = nc.dram_tensor("lhsT_out", [128, 128], f16, kind="ExternalInput")
    lB_d = nc.dram_tensor("lhsT_B", [128, 16], f16, kind="ExternalInput")
    lC_d = nc.dram_tensor("lhsT_C", [16, 64], f32r, kind="ExternalInput")
    bias_d = nc.dram_tensor("biases", [128, 6], f32, kind="ExternalInput")
    dec_d = nc.dram_tensor("decay", [16, HEAD_L], f32, kind="ExternalInput")

    from contextlib import ExitStack

    with TileContext(nc) as tc, ExitStack() as _ctx:
        _p = lambda **kw: _ctx.enter_context(tc.tile_pool(**kw))
        cpool = _p(name="consts", bufs=1)
        xpool = _p(name="xin", bufs=8)
        xppool = _p(name="xp", bufs=12)
        xcpool = _p(name="xc", bufs=12)
        gpool = _p(name="gact", bufs=8)
        ypool = _p(name="ysb", bufs=10)
        hpool = _p(name="headsb", bufs=1)
        pin = _p(name="psum_in", bufs=_CFG["pin"], space="PSUM")
        pout = _p(name="psum_ot", bufs=_CFG["pot"], space="PSUM")
        pdw = _p(name="psum_dw", bufs=_CFG["pdw"], space="PSUM")
        psig_pool = _p(name="psum_sg", bufs=_CFG["psig"], space="PSUM")
        if _CFG["split_tanh"]:
            ptau_pool = _p(name="psum_tu", bufs=_CFG["ptau"], space="PSUM")
        if True:
            # ---- constants into SBUF
            lin = cpool.tile([128, 128], f32r, tag="lin")
            nc.sync.dma_start(lin[:, :], lin_d[:, :])
            ldw = []
            for k in range(3):
                t = cpool.tile([128, 128], f16, tag=f"ldw{k}", name=f"ldw{k}")
                nc.sync.dma_start(t[:, :], ldw_d[k, :, :])
                ldw.append(t)
            lcx = []
            for k in range(3):
                t = cpool.tile([128, 64], f16, tag=f"lcx{k}", name=f"lcx{k}")
                nc.sync.dma_start(t[:, :], lcx_d[k, :, :])
                lcx.append(t)
            lgl = cpool.tile([128, 128], f16, tag="lgl")
            nc.sync.dma_start(lgl[:, :], lgl_d[:, :])
            lgh = cpool.tile([128, 128], f16, tag="lgh")
            nc.sync.dma_start(lgh[:, :], lgh_d[:, :])
            lo = cpool.tile([128, 128], f16, tag="lo")
            nc.sync.dma_start(lo[:, :], lo_d[:, :])
            lB = cpool.tile([128, 16], f16, tag="lB")
            nc.sync.dma_start(lB[:, :], lB_d[:, :])
            lC = cpool.tile([16, 64], f32r, tag="lC")
            nc.sync.dma_start(lC[:, :], lC_d[:, :])
            bias = cpool.tile([128, 6], f32, tag="bias")
            nc.sync.dma_start(bias[:, :], bias_d[:, :])
            b_in = bias[:, 0:1]
            b_conv = bias[:, 1:2]
            bg_l = bias[:, 2:3]
            bg_h = bias[:, 3:4]
            d_ch = bias[:, 4:5]
            b_out = bias[:, 5:6]

            dec = hpool.tile([16, HEAD_L], f32, tag="dec")
            nc.sync.dma_start(dec[:, :], dec_d[:, :])
            bw = hpool.tile([16, HEAD_L], f32, tag="bw")
            wsc = hpool.tile([16, HEAD_L], f32, tag="wsc")
            zer = hpool.tile([16, HEAD_L], f32, tag="zer")
            nc.gpsimd.memset(zer[:, :], 0.0)
            state = hpool.tile([16, HEAD_L], f32r, tag="state")
            spairq = hpool.tile([128, 2 * W], f32, tag="spairq")
            cstar = hpool.tile([128, 1], f32, tag="cstar")

            xp_tiles = {}
            xc_tiles = {}

            def rows3d(dram, j):
                # [(h:2), (c:64), (w:512)] enumeration -> [128, 512] pair tile
                return dram[:, 2 * j : 2 * j + 2, :].rearrange("c h w -> h c w")

            x_tiles = {}

            def load_x(j):
                xt = xpool.tile([128, W], f32r, tag="x")
                nc.sync.dma_start(xt[:, :], rows3d(x_d, j))
                x_tiles[j] = xt

            def inproj_silu(j):
                p = pin.tile([128, W], f32, tag="pin")
                xt = x_tiles.pop(j)
                nc.tensor.matmul(
                    p[:, :], lin[:, :], xt[:, :],
                    start=True, stop=True, skip_group_check=True,
                )
                xpt = xppool.tile([128, W], f16, tag="xp")
                nc.scalar.activation(xpt[:, :], p[:, :], AF.Silu, bias=b_in, scale=1.0)
                xp_tiles[j] = xpt

            def dw_silu(j):
                p = pdw.tile([128, W], f32, tag="pdw")
                tj = xp_tiles[j]
                mms = [
                    dict(out=p[:, _SH[1][0]], lhsT=ldw[1][:, :], rhs=tj[:, _SH[1][1]],
                         start=True),
                    dict(out=p[:, _SH[0][0]], lhsT=ldw[0][:, :], rhs=tj[:, _SH[0][1]]),
                    dict(out=p[:, _SH[2][0]], lhsT=ldw[2][:, :], rhs=tj[:, _SH[2][1]]),
                ]
                if j > 0:
                    tm = xp_tiles[j - 1]   # row 2j-1 lives in parts 64:128
                    for kx in range(3):
                        osl, isl = _SH[kx]
                        mms.append(dict(
                            out=p[0:64, osl], lhsT=lcx[kx][64:128, :],
                            rhs=tm[64:128, isl], tile_position=(64, 0),
                        ))
                if j < 2 * NQ - 1:
                    tp = xp_tiles[j + 1]   # row 2j+2 lives in parts 0:64
                    for kx in range(3):
                        osl, isl = _SH[kx]
                        mms.append(dict(
                            out=p[64:128, osl], lhsT=lcx[kx][0:64, :],
                            rhs=tp[0:64, isl], tile_position=(0, 64),
                        ))
                last = len(mms) - 1
                for i, mm in enumerate(mms):
                    nc.tensor.matmul(
                        mm["out"], mm["lhsT"], mm["rhs"],
                        start=mm.get("start", False), stop=(i == last),
                        tile_position=mm.get("tile_position"),
                        skip_group_check=True,
                    )
                xct = xcpool.tile([128, W], f16, tag="xc")
                nc.scalar.activation(xct[:, :], p[:, :], AF.Silu, bias=b_conv, scale=1.0)
                xc_tiles[j] = xct

            gate_psums = {}
            out_state = {}

            def gate_mms(j):
                tj = xp_tiles[j]
                if _CFG["split_tanh"]:
                    ps = psig_pool.tile([128, W], f32, tag="psg")
                    nc.tensor.matmul(
                        ps[:, :], lgl[:, :], tj[:, :],
                        start=True, stop=True, skip_group_check=True,
                    )
                    pt = ptau_pool.tile([128, W], f32, tag="ptu")
                    nc.tensor.matmul(
                        pt[:, :], lgh[:, :], tj[:, :],
                        start=True, stop=True, skip_group_check=True,
                    )
                    gate_psums[j] = (ps, pt)
                else:
                    pgt = psig_pool.tile([128, 2 * W], f32, tag="psg")
                    nc.tensor.matmul(
                        pgt[:, 0:W], lgl[:, :], tj[:, :],
                        start=True, stop=True, skip_group_check=True,
                    )
                    nc.tensor.matmul(
                        pgt[:, W : 2 * W], lgh[:, :], tj[:, :],
                        start=True, stop=True, skip_group_check=True,
                    )
                    gate_psums[j] = pgt

            def act_dve_phase(j, head):
                pg_j = gate_psums.pop(j)
                xct = xc_tiles.pop(j)
                tgt = gpool.tile([128, 2 * W], f16, tag="tgt")
                if _CFG["split_tanh"]:
                    ps, pt = pg_j
                    if zero_gate_bias:
                        nc.scalar.activation(tgt[:, 0:W], ps[:, :], AF.Tanh)
                        nc.scalar.activation(tgt[:, W : 2 * W], pt[:, :], AF.Tanh)
                    else:
                        nc.scalar.activation(
                            tgt[:, 0:W], ps[:, :], AF.Tanh, bias=bg_l, scale=0.5
                        )
                        nc.scalar.activation(
                            tgt[:, W : 2 * W], pt[:, :], AF.Tanh, bias=bg_h
                        )
                elif zero_gate_bias:
                    nc.scalar.activation(tgt[:, :], pg_j[:, :], AF.Tanh)
                else:
                    nc.scalar.activation(
                        tgt[:, 0:W], pg_j[:, 0:W], AF.Tanh, bias=bg_l, scale=0.5
                    )
                    nc.scalar.activation(
                        tgt[:, W : 2 * W], pg_j[:, W : 2 * W], AF.Tanh, bias=bg_h
                    )
                sm = gpool.tile([128, W], f16, tag="sm")
                if head:
                    nc.vector.scalar_tensor_tensor(
                        sm[:, :], xct[:, :], d_ch, spairq[:, j * W : j * W + W],
                        op0=ALU.mult, op1=ALU.add,
                    )
                else:
                    nc.vector.tensor_scalar(
                        sm[:, :], xct[:, :], d_ch, cstar[:, 0:1],
                        op0=ALU.mult, op1=ALU.add,
                    )
                u = gpool.tile([128, W], f16, tag="u")
                nc.vector.tensor_tensor(u[:, :], sm[:, :], tgt[:, W : 2 * W], op=ALU.add)
                g = gpool.tile([128, W], f16, tag="g")
                nc.vector.scalar_tensor_tensor(
                    g[:, :], tgt[:, 0:W], 1.0, u[:, :], op0=ALU.add, op1=ALU.mult
                )
                out_state[j] = g

            pot_state = {}

            def out_mm(j):
                g = out_state.pop(j)
                pot = pout.tile([128, W], f32, tag="pot")
                nc.tensor.matmul(
                    pot[:, :], lo[:, :], g[:, :],
                    start=True, stop=True, skip_group_check=True,
                )
                pot_state[j] = pot

            def y_finish(j):
                pot = pot_state.pop(j)
                yt = ypool.tile([128, W], f32, tag="y")
                nc.vector.tensor_scalar(yt[:, :], pot[:, :], b_out, None, op0=ALU.add)
                nc.gpsimd.dma_start(rows3d(y_d, j), yt[:, :])

            # ---- head bootstrap: rows 0..3 (pairs 0,1) need the real scan
            NPAIR = 2 * NQ
            for j in range(6):
                load_x(j)
            for j in range(4):
                inproj_silu(j)
            dw_silu(0)
            dw_silu(1)
            for r in range(4):
                jj, rr = divmod(r, 2)   # jj = pair, rr = row in pair
                pb = pin.tile([16, W], f32, tag="pin", name=f"pb{r}")
                nc.tensor.matmul(
                    pb[:, :], lB[64 * rr : 64 * rr + 64, :],
                    xc_tiles[jj][64 * rr : 64 * rr + 64, :],
                    start=True, stop=True, tile_position=(64 * rr, 0),
                    skip_group_check=True,
                )
                nc.vector.tensor_copy(bw[:, W * r : W * (r + 1)], pb[:, :])
            nc.vector.tensor_tensor(wsc[:, :], bw[:, :], dec[:, :], op=ALU.mult)
            nc.vector.tensor_tensor_scan(
                state[:, :], zer[:, :], wsc[:, :], initial=0.0,
                op0=ALU.add, op1=ALU.add,
            )
            for r in range(4):
                jj, rr = divmod(r, 2)
                pc = pout.tile([64, W], f32, tag="pot", name=f"pc{r}")
                nc.tensor.matmul(
                    pc[:, :], lC[:, :], state[:, W * r : W * (r + 1)],
                    start=True, stop=True, skip_group_check=True,
                )
                nc.scalar.copy(
                    spairq[64 * rr : 64 * rr + 64, jj * W : jj * W + W], pc[:, :]
                )
            # c* = (wC @ state)[:, HEAD_L-1] = spairq[64:128, 1023], both halves
            nc.sync.dma_start(cstar[0:64, 0:1], spairq[64:128, 2 * W - 1 : 2 * W])
            nc.sync.dma_start(cstar[64:128, 0:1], spairq[64:128, 2 * W - 1 : 2 * W])
            gate_mms(0)

            # ---- main software-pipelined loop
            # iteration j: DMA j+6, in_proj j+4, dw j+2, gate-MMs j+1,
            # tanh+DVE j, out-proj MM j-1, ybias j-1-lag
            lag = _CFG["ybias_lag"]
            for j in range(NPAIR + 2 + lag):
                if j + 6 < NPAIR:
                    load_x(j + 6)
                if 4 <= j + 4 < NPAIR:
                    inproj_silu(j + 4)
                if lag >= 1 and 0 <= j - 1 - lag < NPAIR:
                    y_finish(j - 1 - lag)
                if 0 <= j - 1 < NPAIR:
                    out_mm(j - 1)
                if lag == 0 and 0 <= j - 1 < NPAIR:
                    y_finish(j - 1)
                if j < NPAIR:
                    act_dve_phase(j, head=(j < 2))
                if 2 <= j + 2 < NPAIR:
                    dw_silu(j + 2)
                if 1 <= j + 1 < NPAIR:
                    gate_mms(j + 1)

    _split_waits(nc, 1)
    return nc


def _prep_consts(inputs):
    fp = np.float32
    s = fp(1.0) / np.sqrt(fp(1.0) + fp(BN_EPS))

    g_in = inputs["g_in"].astype(fp); b_in = inputs["b_in"].astype(fp)
    g_conv = inputs["g_conv"].astype(fp); b_conv = inputs["b_conv"].astype(fp)
    g_gate = inputs["g_gate"].astype(fp); b_gate = inputs["b_gate"].astype(fp)
    g_out = inputs["g_out"].astype(fp); b_out = inputs["b_out"].astype(fp)

    def blockdiag2(m):
        z = np.zeros((128, 128), m.dtype)
        z[0:64, 0:64] = m
        z[64:128, 64:128] = m
        return z

    w_in = (g_in * s)[:, None] * inputs["w_in"].astype(fp)
    lhsT_in = blockdiag2(np.ascontiguousarray(w_in.T))

    wdw = inputs["w_dw"].astype(fp)[:, 0] * (g_conv * s)[:, None, None]
    idx = np.arange(64)
    lhsT_dw = np.zeros((3, 128, 128), fp)
    lhsT_cx = np.zeros((3, 128, 64), fp)
    for kx in range(3):
        lhsT_dw[kx, idx, idx] = wdw[:, 1, kx]
        lhsT_dw[kx, idx + 64, idx] = wdw[:, 2, kx]
        lhsT_dw[kx, idx, idx + 64] = wdw[:, 0, kx]
        lhsT_dw[kx, idx + 64, idx + 64] = wdw[:, 1, kx]
        lhsT_cx[kx, idx, idx] = wdw[:, 2, kx]
        lhsT_cx[kx, idx + 64, idx] = wdw[:, 0, kx]

    zero_gate_bias = not np.any(b_gate)
    w_g = (g_gate * s)[:, None] * inputs["w_gate"].astype(fp)
    glo = w_g[0:64] * fp(0.5) if zero_gate_bias else w_g[0:64]
    lhsT_gl = blockdiag2(np.ascontiguousarray(glo.T))
    lhsT_gh = blockdiag2(np.ascontiguousarray(w_g[64:128].T))

    w_out = fp(0.5) * (g_out * s)[:, None] * inputs["w_out"].astype(fp)
    lhsT_out = blockdiag2(np.ascontiguousarray(w_out.T))

    wB_T = np.ascontiguousarray(inputs["wB"].astype(fp).T)
    lhsT_B = np.concatenate([wB_T, wB_T], axis=0)
    lhsT_C = np.ascontiguousarray(inputs["wC"].astype(fp).T)

    d_ch = inputs["D"].astype(fp)[0, :, 0, 0]
    biases = np.zeros((128, 6), fp)
    biases[:, 0] = np.tile(b_in, 2)
    biases[:, 1] = np.tile(b_conv, 2)
    biases[:, 2] = np.tile(fp(0.5) * b_gate[0:64], 2)
    biases[:, 3] = np.tile(b_gate[64:128], 2)
    biases[:, 4] = np.tile(d_ch, 2)
    biases[:, 5] = np.tile(b_out, 2)
    if not zero_gate_bias:
        # gate half not pre-scaled; sigmoid arg is (z+b)/2 via ACT scale=0.5
        # (handled by the two-op path; bias cols stay as above)
        pass

    a_vec = inputs["A"].astype(fp)[0, :, 0]
    k = np.arange(HEAD_L, dtype=fp)
    decay = np.exp(a_vec[:, None] * k[None, :]).astype(fp)
    tail = np.exp(a_vec.astype(fp) * fp(HEAD_L))
    if not np.all(tail == 0.0):
        raise NotImplementedError(
            "decay does not underflow within the head region; enlarge HEAD_L"
        )

    return {
        "lhsT_in": lhsT_in,
        "lhsT_dw": lhsT_dw.astype(np.float16),
        "lhsT_cx": lhsT_cx.astype(np.float16),
        "lhsT_gl": lhsT_gl.astype(np.float16),
        "lhsT_gh": lhsT_gh.astype(np.float16),
        "lhsT_out": lhsT_out.astype(np.float16),
        "lhsT_B": lhsT_B.astype(np.float16),
        "lhsT_C": lhsT_C,
        "biases": biases,
        "decay": decay,
    }, zero_gate_bias


_progs = {}


def _get_prog(zero_gate_bias=True):
    if zero_gate_bias not in _progs:
        _progs[zero_gate_bias] = _build_program(zero_gate_bias)
    return _progs[zero_gate_bias]


def kernel(**inputs):
    consts, zgb = _prep_consts(inputs)
    nc = _get_prog(zgb)
    x = np.ascontiguousarray(inputs["x"].astype(np.float32))
    in_maps = [dict(consts, x=x[b]) for b in range(B)]
    res = bass_utils.run_bass_kernel_spmd(nc, in_maps, core_ids=list(range(N_CORES)))
    y = np.stack([res.results[b]["y"] for b in range(B)], axis=0)
    return y.astype(np.float32)


# revision 58
# speedup vs baseline: 22002.3689x; 22002.3689x over previous
"""Trainium2 Bass kernel for CustomMamba2D.

Sharding: data-parallel over batch across 8 NeuronCores (B=8 -> 1 image/core).

Per-core layout: the 64ch x 512x512 image is processed in 256 row-PAIRS.
A pair tile is [128 partitions, 512 free]: partitions = (row-in-pair r,
channel c) = 2*64, free = w.  All channel-mixing matmuls use 128-partition
operands with block-diagonal weights.  The depthwise 3x3 conv is 9 PE
matmuls per pair with diagonal weight blocks; the 6 "cross" row taps use
64-partition operands packed into disjoint PE quadrants via tile_position
(concurrent on HW).  dx shifts are free-dim AP offsets (narrower
accumulation range = zero pad).  The whole thing is software-pipelined:
DMA 6 pairs ahead, in_proj 4 ahead, depthwise 2 ahead, out-proj 1 behind,
with PSUM pool depths tuned against the TimelineSim cost model.

SSM shortcut: exp(A*k) underflows to exactly 0.0 (fp32) for k >= ~1040, so
the reference cumsum is bitwise constant from l = 2048 on.  Only rows 0..3
need the scan (DVE tensor_tensor_scan); all other positions use the
constant c* = wC @ state[:, 2047].

sigmoid(g) = (tanh(g/2)+1)/2 with the 0.5 folded into the gate weights and
w_out, so ALL transcendentals (SiLU x2, tanh) live in one ACT table set
(silu_and_others -> one table load).  BN scales fold into weights host-side.
Intermediates are fp16 (DVE 2x/4x modes); matmuls on fp32 data use float32r
(1 cycle/col at free-dim >= 256).  Output DMAs ride the otherwise-idle
GPSIMD SWDGE queue so the SP sequencer only pays for input DMAs.
"""

import os
import sys

for _p in (
    "/root/.axon_site",
    "/root/.axon_site/_ro/trn_rl_repo",
    "/root/.axon_site/_ro/pypackages",
    "/opt/trn_rl_repo",
    "/opt/pypackages",
):
    if os.path.isdir(_p) and _p not in sys.path:
        sys.path.append(_p)

import numpy as np

import concourse.bass as bass
import concourse.mybir as mybir
from concourse.tile import TileContext
from concourse import bass_utils

f32 = mybir.dt.float32
f32r = mybir.dt.float32r
f16 = mybir.dt.float16
AF = mybir.ActivationFunctionType
ALU = mybir.AluOpType

D_MODEL, D_STATE, D_CONV, D_INNER = 64, 16, 3, 64
BN_EPS = 1e-5
B, H, W = 8, 512, 512
NQ = H // 4            # 128 row quads (4 rows each)
HEAD_L = 2048          # rows 0..3 carry the live part of the scan
N_CORES = 8

# (out_slice, in_slice) per kx = dx+1, within one 512-wide pair block
_SH = (
    (slice(1, W), slice(0, W - 1)),
    (slice(0, W), slice(0, W)),
    (slice(0, W - 1), slice(1, W)),
)


def _off(sl, base):
    return slice(sl.start + base, sl.stop + base)


def _split_waits(nc, maxw=1):
    """This walrus build encodes at most ONE sync wait per instruction and
    refuses to split multi-wait instructions itself.  Move extra waits onto
    NoOp carriers inserted just before the owning instruction (same engine,
    so ordering is preserved)."""
    for fn in nc.m.functions:
        for bb in fn.blocks:
            out, changed = [], False
            for inst in bb.instructions:
                si = inst.sync_info
                if si is not None and len(si.on_wait) > maxw:
                    waits = list(si.on_wait)
                    for k, wt in enumerate(waits[maxw:]):
                        out.append(
                            mybir.InstNoOp(
                                name=f"{inst.name}_sw{k}",
                                engine=inst.engine,
                                bass_nofuse=True,
                                sync_info=mybir.SyncInfo(on_wait=[wt], on_update=[]),
                            )
                        )
                    inst.sync_info = mybir.SyncInfo(
                        on_wait=waits[:maxw], on_update=list(si.on_update)
                    )
                    changed = True
                out.append(inst)
            if changed:
                bb.instructions = out


# PSUM/schedule configuration (sweepable via TimelineSim):
#   split_tanh: two [128,512] gate psums + two tanh ops instead of one
#       merged [128,1024] psum + single tanh
#   ybias_lag: iterations between out-proj MM and the ybias DVE read
_CFG = dict(split_tanh=True, pin=1, pot=1, pdw=3, psig=1, ptau=2, ybias_lag=0)


def _build_program(zero_gate_bias):
    nc = bass.Bass("TRN2", target_bir_lowering=False, debug=False, num_devices=N_CORES)

    x_d = nc.dram_tensor("x", [D_MODEL, H, W], f32r, kind="ExternalInput")
    y_d = nc.dram_tensor("y", [D_MODEL, H, W], f32, kind="ExternalOutput")
    lin_d = nc.dram_tensor("lhsT_in", [128, 128], f32r, kind="ExternalInput")
    ldw_d = nc.dram_tensor("lhsT_dw", [3, 128, 128], f16, kind="ExternalInput")
    lcx_d = nc.dram_tensor("lhsT_cx", [3, 128, 64], f16, kind="ExternalInput")
    lgl_d = nc.dram_tensor("lhsT_gl", [128, 128], f16, kind="ExternalInput")
    lgh_d = nc.dram_tensor("lhsT_gh", [128, 128], f16, kind="ExternalInput")
    lo_d = nc.dram_tensor("lhsT_out", [128, 128], f16, kind="ExternalInput")
    lB_d = nc.dram_tensor("lhsT_B", [128, 16], f16, kind="ExternalInput")
    lC_d = nc.dram_tensor("lhsT_C", [16, 64], f32r, kind="ExternalInput")
    bias_d = nc.dram_tensor("biases", [128, 6], f32, kind="ExternalInput")
    dec_d = nc.dram_tensor("decay", [16, HEAD_L], f32, kind="ExternalInput")

    from contextlib import ExitStack

    with TileContext(nc) as tc, ExitStack() as _ctx:
        _p = lambda **kw: _ctx.enter_context(tc.tile_pool(**kw))
        cpool = _p(name="consts", bufs=1)
        xpool = _p(name="xin", bufs=8)
        xppool = _p(name="xp", bufs=12)
        xcpool = _p(name="xc", bufs=12)
        gpool = _p(name="gact", bufs=8)
        ypool = _p(name="ysb", bufs=10)
        hpool = _p(name="headsb", bufs=1)
        pin = _p(name="psum_in", bufs=_CFG["pin"], space="PSUM")
        pout = _p(name="psum_ot", bufs=_CFG["pot"], space="PSUM")
        pdw = _p(name="psum_dw", bufs=_CFG["pdw"], space="PSUM")
        psig_pool = _p(name="psum_sg", bufs=_CFG["psig"], space="PSUM")
        if _CFG["split_tanh"]:
            ptau_pool = _p(name="psum_tu", bufs=_CFG["ptau"], space="PSUM")
        if True:
            # ---- constants into SBUF
            lin = cpool.tile([128, 128], f32r, tag="lin")
            nc.sync.dma_start(lin[:, :], lin_d[:, :])
            ldw = []
            for k in range(3):
                t = cpool.tile([128, 128], f16, tag=f"ldw{k}", name=f"ldw{k}")
                nc.sync.dma_start(t[:, :], ldw_d[k, :, :])
                ldw.append(t)
            lcx = []
            for k in range(3):
                t = cpool.tile([128, 64], f16, tag=f"lcx{k}", name=f"lcx{k}")
                nc.sync.dma_start(t[:, :], lcx_d[k, :, :])
                lcx.append(t)
            lgl = cpool.tile([128, 128], f16, tag="lgl")
            nc.sync.dma_start(lgl[:, :], lgl_d[:, :])
            lgh = cpool.tile([128, 128], f16, tag="lgh")
            nc.sync.dma_start(lgh[:, :], lgh_d[:, :])
            lo = cpool.tile([128, 128], f16, tag="lo")
            nc.sync.dma_start(lo[:, :], lo_d[:, :])
            lB = cpool.tile([128, 16], f16, tag="lB")
            nc.sync.dma_start(lB[:, :], lB_d[:, :])
            lC = cpool.tile([16, 64], f32r, tag="lC")
            nc.sync.dma_start(lC[:, :], lC_d[:, :])
            bias = cpool.tile([128, 6], f32, tag="bias")
            nc.sync.dma_start(bias[:, :], bias_d[:, :])
            b_in = bias[:, 0:1]
            b_conv = bias[:, 1:2]
            bg_l = bias[:, 2:3]
            bg_h = bias[:, 3:4]
            d_ch = bias[:, 4:5]
            b_out = bias[:, 5:6]

            dec = hpool.tile([16, HEAD_L], f32, tag="dec")
            nc.sync.dma_start(dec[:, :], dec_d[:, :])
            bw = hpool.tile([16, HEAD_L], f32, tag="bw")
            wsc = hpool.tile([16, HEAD_L], f32, tag="wsc")
            zer = hpool.tile([16, HEAD_L], f32, tag="zer")
            nc.gpsimd.memset(zer[:, :], 0.0)
            state = hpool.tile([16, HEAD_L], f32r, tag="state")
            spairq = hpool.tile([128, 2 * W], f32, tag="spairq")
            cstar = hpool.tile([128, 1], f32, tag="cstar")

            xp_tiles = {}
            xc_tiles = {}

            def rows3d(dram, j):
                # [(h:2), (c:64), (w:512)] enumeration -> [128, 512] pair tile
                return dram[:, 2 * j : 2 * j + 2, :].rearrange("c h w -> h c w")

            x_tiles = {}

            def load_x(j):
                xt = xpool.tile([128, W], f32r, tag="x")
                nc.sync.dma_start(xt[:, :], rows3d(x_d, j))
                x_tiles[j] = xt

            def inproj_silu(j):
                p = pin.tile([128, W], f32, tag="pin")
                xt = x_tiles.pop(j)
                nc.tensor.matmul(
                    p[:, :], lin[:, :], xt[:, :],
                    start=True, stop=True, skip_group_check=True,
                )
                xpt = xppool.tile([128, W], f16, tag="xp")
                nc.scalar.activation(xpt[:, :], p[:, :], AF.Silu, bias=b_in, scale=1.0)
                xp_tiles[j] = xpt

            def dw_silu(j):
                p = pdw.tile([128, W], f32, tag="pdw")
                tj = xp_tiles[j]
                mms = [
                    dict(out=p[:, _SH[1][0]], lhsT=ldw[1][:, :], rhs=tj[:, _SH[1][1]],
                         start=True),
                    dict(out=p[:, _SH[0][0]], lhsT=ldw[0][:, :], rhs=tj[:, _SH[0][1]]),
                    dict(out=p[:, _SH[2][0]], lhsT=ldw[2][:, :], rhs=tj[:, _SH[2][1]]),
                ]
                if j > 0:
                    tm = xp_tiles[j - 1]   # row 2j-1 lives in parts 64:128
                    for kx in range(3):
                        osl, isl = _SH[kx]
                        mms.append(dict(
                            out=p[0:64, osl], lhsT=lcx[kx][64:128, :],
                            rhs=tm[64:128, isl], tile_position=(64, 0),
                        ))
                if j < 2 * NQ - 1:
                    tp = xp_tiles[j + 1]   # row 2j+2 lives in parts 0:64
                    for kx in range(3):
                        osl, isl = _SH[kx]
                        mms.append(dict(
                            out=p[64:128, osl], lhsT=lcx[kx][0:64, :],
                            rhs=tp[0:64, isl], tile_position=(0, 64),
                        ))
                last = len(mms) - 1
                for i, mm in enumerate(mms):
                    nc.tensor.matmul(
                        mm["out"], mm["lhsT"], mm["rhs"],
                        start=mm.get("start", False), stop=(i == last),
                        tile_position=mm.get("tile_position"),
                        skip_group_check=True,
                    )
                xct = xcpool.tile([128, W], f16, tag="xc")
                nc.scalar.activation(xct[:, :], p[:, :], AF.Silu, bias=b_conv, scale=1.0)
                xc_tiles[j] = xct

            gate_psums = {}
            out_state = {}

            def gate_mms(j):
                tj = xp_tiles[j]
                if _CFG["split_tanh"]:
                    ps = psig_pool.tile([128, W], f32, tag="psg")
                    nc.tensor.matmul(
                        ps[:, :], lgl[:, :], tj[:, :],
                        start=True, stop=True, skip_group_check=True,
                    )
                    pt = ptau_pool.tile([128, W], f32, tag="ptu")
                    nc.tensor.matmul(
                        pt[:, :], lgh[:, :], tj[:, :],
                        start=True, stop=True, skip_group_check=True,
                    )
                    gate_psums[j] = (ps, pt)
                else:
                    pgt = psig_pool.tile([128, 2 * W], f32, tag="psg")
                    nc.tensor.matmul(
                        pgt[:, 0:W], lgl[:, :], tj[:, :],
                        start=True, stop=True, skip_group_check=True,
                    )
                    nc.tensor.matmul(
                        pgt[:, W : 2 * W], lgh[:, :], tj[:, :],
                        start=True, stop=True, skip_group_check=True,
                    )
                    gate_psums[j] = pgt

            def act_dve_phase(j, head):
                pg_j = gate_psums.pop(j)
                xct = xc_tiles.pop(j)
                tgt = gpool.tile([128, 2 * W], f16, tag="tgt")
                if _CFG["split_tanh"]:
                    ps, pt = pg_j
                    if zero_gate_bias:
                        nc.scalar.activation(tgt[:, 0:W], ps[:, :], AF.Tanh)
                        nc.scalar.activation(tgt[:, W : 2 * W], pt[:, :], AF.Tanh)
                    else:
                        nc.scalar.activation(
                            tgt[:, 0:W], ps[:, :], AF.Tanh, bias=bg_l, scale=0.5
                        )
                        nc.scalar.activation(
                            tgt[:, W : 2 * W], pt[:, :], AF.Tanh, bias=bg_h
                        )
                elif zero_gate_bias:
                    nc.scalar.activation(tgt[:, :], pg_j[:, :], AF.Tanh)
                else:
                    nc.scalar.activation(
                        tgt[:, 0:W], pg_j[:, 0:W], AF.Tanh, bias=bg_l, scale=0.5
                    )
                    nc.scalar.activation(
                        tgt[:, W : 2 * W], pg_j[:, W : 2 * W], AF.Tanh, bias=bg_h
                    )
                sm = gpool.tile([128, W], f16, tag="sm")
                if head:
                    nc.vector.scalar_tensor_tensor(
                        sm[:, :], xct[:, :], d_ch, spairq[:, j * W : j * W + W],
                        op0=ALU.mult, op1=ALU.add,
                    )
                else:
                    nc.vector.tensor_scalar(
                        sm[:, :], xct[:, :], d_ch, cstar[:, 0:1],
                        op0=ALU.mult, op1=ALU.add,
                    )
                u = gpool.tile([128, W], f16, tag="u")
                nc.vector.tensor_tensor(u[:, :], sm[:, :], tgt[:, W : 2 * W], op=ALU.add)
                g = gpool.tile([128, W], f16, tag="g")
                nc.vector.scalar_tensor_tensor(
                    g[:, :], tgt[:, 0:W], 1.0, u[:, :], op0=ALU.add, op1=ALU.mult
                )
                out_state[j] = g

            pot_state = {}

            def out_mm(j):
                g = out_state.pop(j)
                pot = pout.tile([128, W], f32, tag="pot")
                nc.tensor.matmul(
                    pot[:, :], lo[:, :], g[:, :],
                    start=True, stop=True, skip_group_check=True,
                )
                pot_state[j] = pot

            def y_finish(j):
                pot = pot_state.pop(j)
                yt = ypool.tile([128, W], f32, tag="y")
                nc.vector.tensor_scalar(yt[:, :], pot[:, :], b_out, None, op0=ALU.add)
                nc.gpsimd.dma_start(rows3d(y_d, j), yt[:, :])

            # ---- head bootstrap: rows 0..3 (pairs 0,1) need the real scan
            NPAIR = 2 * NQ
            for j in range(6):
                load_x(j)
            for j in range(4):
                inproj_silu(j)
            dw_silu(0)
            dw_silu(1)
            for r in range(4):
                jj, rr = divmod(r, 2)   # jj = pair, rr = row in pair
                pb = pin.tile([16, W], f32, tag="pin", name=f"pb{r}")
                nc.tensor.matmul(
                    pb[:, :], lB[64 * rr : 64 * rr + 64, :],
                    xc_tiles[jj][64 * rr : 64 * rr + 64, :],
                    start=True, stop=True, tile_position=(64 * rr, 0),
                    skip_group_check=True,
                )
                nc.vector.tensor_copy(bw[:, W * r : W * (r + 1)], pb[:, :])
            nc.vector.tensor_tensor(wsc[:, :], bw[:, :], dec[:, :], op=ALU.mult)
            nc.vector.tensor_tensor_scan(
                state[:, :], zer[:, :], wsc[:, :], initial=0.0,
                op0=ALU.add, op1=ALU.add,
            )
            for r in range(4):
                jj, rr = divmod(r, 2)
                pc = pout.tile([64, W], f32, tag="pot", name=f"pc{r}")
                nc.tensor.matmul(
                    pc[:, :], lC[:, :], state[:, W * r : W * (r + 1)],
                    start=True, stop=True, skip_group_check=True,
                )
                nc.scalar.copy(
                    spairq[64 * rr : 64 * rr + 64, jj * W : jj * W + W], pc[:, :]
                )
            # c* = (wC @ state)[:, HEAD_L-1] = spairq[64:128, 1023], both halves
            nc.sync.dma_start(cstar[0:64, 0:1], spairq[64:128, 2 * W - 1 : 2 * W])
            nc.sync.dma_start(cstar[64:128, 0:1], spairq[64:128, 2 * W - 1 : 2 * W])
            gate_mms(0)

            # ---- main software-pipelined loop
            # iteration j: DMA j+6, in_proj j+4, dw j+2, gate-MMs j+1,
            # tanh+DVE j, out-proj MM j-1, ybias j-1-lag
            lag = _CFG["ybias_lag"]
            for j in range(NPAIR + 2 + lag):
                if j < NPAIR:
                    act_dve_phase(j, head=(j < 2))
                if j + 6 < NPAIR:
                    load_x(j + 6)
                if 4 <= j + 4 < NPAIR:
                    inproj_silu(j + 4)
                if lag >= 1 and 0 <= j - 1 - lag < NPAIR:
                    y_finish(j - 1 - lag)
                if 0 <= j - 1 < NPAIR:
                    out_mm(j - 1)
                if lag == 0 and 0 <= j - 1 < NPAIR:
                    y_finish(j - 1)
                if 2 <= j + 2 < NPAIR:
                    dw_silu(j + 2)
                if 1 <= j + 1 < NPAIR:
                    gate_mms(j + 1)

    _split_waits(nc, 1)
    return nc


def _prep_consts(inputs):
    fp = np.float32
    s = fp(1.0) / np.sqrt(fp(1.0) + fp(BN_EPS))

    g_in = inputs["g_in"].astype(fp); b_in = inputs["b_in"].astype(fp)
    g_conv = inputs["g_conv"].astype(fp); b_conv = inputs["b_conv"].astype(fp)
    g_gate = inputs["g_gate"].astype(fp); b_gate = inputs["b_gate"].astype(fp)
    g_out = inputs["g_out"].astype(fp); b_out = inputs["b_out"].astype(fp)

    def blockdiag2(m):
        z = np.zeros((128, 128), m.dtype)
        z[0:64, 0:64] = m
        z[64:128, 64:128] = m
        return z

    w_in = (g_in * s)[:, None] * inputs["w_in"].astype(fp)
    lhsT_in = blockdiag2(np.ascontiguousarray(w_in.T))

    wdw = inputs["w_dw"].astype(fp)[:, 0] * (g_conv * s)[:, None, None]
    idx = np.arange(64)
    lhsT_dw = np.zeros((3, 128, 128), fp)
    lhsT_cx = np.zeros((3, 128, 64), fp)
    for kx in range(3):
        lhsT_dw[kx, idx, idx] = wdw[:, 1, kx]
        lhsT_dw[kx, idx + 64, idx] = wdw[:, 2, kx]
        lhsT_dw[kx, idx, idx + 64] = wdw[:, 0, kx]
        lhsT_dw[kx, idx + 64, idx + 64] = wdw[:, 1, kx]
        lhsT_cx[kx, idx, idx] = wdw[:, 2, kx]
        lhsT_cx[kx, idx + 64, idx] = wdw[:, 0, kx]

    zero_gate_bias = not np.any(b_gate)
    w_g = (g_gate * s)[:, None] * inputs["w_gate"].astype(fp)
    glo = w_g[0:64] * fp(0.5) if zero_gate_bias else w_g[0:64]
    lhsT_gl = blockdiag2(np.ascontiguousarray(glo.T))
    lhsT_gh = blockdiag2(np.ascontiguousarray(w_g[64:128].T))

    w_out = fp(0.5) * (g_out * s)[:, None] * inputs["w_out"].astype(fp)
    lhsT_out = blockdiag2(np.ascontiguousarray(w_out.T))

    wB_T = np.ascontiguousarray(inputs["wB"].astype(fp).T)
    lhsT_B = np.concatenate([wB_T, wB_T], axis=0)
    lhsT_C = np.ascontiguousarray(inputs["wC"].astype(fp).T)

    d_ch = inputs["D"].astype(fp)[0, :, 0, 0]
    biases = np.zeros((128, 6), fp)
    biases[:, 0] = np.tile(b_in, 2)
    biases[:, 1] = np.tile(b_conv, 2)
    biases[:, 2] = np.tile(fp(0.5) * b_gate[0:64], 2)
    biases[:, 3] = np.tile(b_gate[64:128], 2)
    biases[:, 4] = np.tile(d_ch, 2)
    biases[:, 5] = np.tile(b_out, 2)
    if not zero_gate_bias:
        # gate half not pre-scaled; sigmoid arg is (z+b)/2 via ACT scale=0.5
        # (handled by the two-op path; bias cols stay as above)
        pass

    a_vec = inputs["A"].astype(fp)[0, :, 0]
    k = np.arange(HEAD_L, dtype=fp)
    decay = np.exp(a_vec[:, None] * k[None, :]).astype(fp)
    tail = np.exp(a_vec.astype(fp) * fp(HEAD_L))
    if not np.all(tail == 0.0):
        raise NotImplementedError(
            "decay does not underflow within the head region; enlarge HEAD_L"
        )

    return {
        "lhsT_in": lhsT_in,
        "lhsT_dw": lhsT_dw.astype(np.float16),
        "lhsT_cx": lhsT_cx.astype(np.float16),
        "lhsT_gl": lhsT_gl.astype(np.float16),
        "lhsT_gh": lhsT_gh.astype(np.float16),
        "lhsT_out": lhsT_out.astype(np.float16),
        "lhsT_B": lhsT_B.astype(np.float16),
        "lhsT_C": lhsT_C,
        "biases": biases,
        "decay": decay,
    }, zero_gate_bias


_progs = {}


def _get_prog(zero_gate_bias=True):
    if zero_gate_bias not in _progs:
        _progs[zero_gate_bias] = _build_program(zero_gate_bias)
    return _progs[zero_gate_bias]


def kernel(**inputs):
    consts, zgb = _prep_consts(inputs)
    nc = _get_prog(zgb)
    x = np.ascontiguousarray(inputs["x"].astype(np.float32))
    in_maps = [dict(consts, x=x[b]) for b in range(B)]
    res = bass_utils.run_bass_kernel_spmd(nc, in_maps, core_ids=list(range(N_CORES)))
    y = np.stack([res.results[b]["y"] for b in range(B)], axis=0)
    return y.astype(np.float32)
